# revision 4
# baseline (speedup 1.0000x reference)
"""Trainium2 Bass kernel for the Clos-factorized MLP (nn_Clos_34282428956960).

The reference network
    h = x.reshape(b, c, 64, 64)                    # [b,c,n,r]
    h = einsum('bcnr,nrm->bcmr', h, w1) + bias1
    h = einsum('bcmr,rmn->bcnm', h, w2) + bias2
    h = einsum('bcnm,mro->bcor', h, w3) + bias3    # contracts BOTH n and m!
    y = h.reshape(b, c, -1)
collapses algebraically: the last einsum sums h over n, so w2 can be
pre-reduced over its output axis (w2s[r,m] = sum_n w2[r,m,n]) and folded
into w1.  The whole network becomes a rank-256 linear map:

    G = X @ W1f + c2      X: [T,4096], W1f[d=(n,r), m] = w1[n,r,m]*w2s[r,m]
    Y = G @ W3f + c3      W3f[m, o*64+r] = w3[m,r,o]
    c2 = bias1 @ w2s + 64*bias2;  c3[o*64+r] = bias3[r]  (period-64)

Per core (tokens sharded 8 ways): X [1024, 4096] -> Y [1024, 4096].

On-chip dataflow per 512-token chunk:
  x tiles [128t, d] --PE transpose--> X^T [128d, t] (SBUF, fp32r)
  MM1: G^T[m_p, t] += W1f[d_p, m].T @ X^T[d_p, t]   (32 d-tiles)
       + c2 via a K=1 matmul (outer product c2 x ones) in the same group
  MM2: Y[t_p, j]  += G^T[m_p, t].T @ W3f[m_p, j]    (2 m-tiles)
  c3 added by DVE during the PSUM->SBUF move (c3 has period 64, one
  [128,512] replica tile serves every output column tile), then 2MB DMA out.

fp32r (reduced-precision fp32 matmul, ~TF32 accuracy, full PE rate at
moving-dim >= 256) is used on the matmul path; accumulation is fp32.
"""

import numpy as np

TOK_TOTAL = 8192          # b*c = 2*4096 tokens
N_CORES = 8
TOK = TOK_TOTAL // N_CORES  # 1024 tokens per core
D = 4096                  # input features
M = 256                   # bottleneck
J = 4096                  # output features
ND = D // 128             # 32 d-tiles
CHUNK = 512               # tokens per MM1 chunk
NCH = TOK // CHUNK        # 2 chunks per core
TPC = CHUNK // 128        # 4 token-tiles per chunk
JT = 512                  # output column tile
NJ = J // JT              # 8 j-tiles

_CACHE = {}


def _build_nc():
    import concourse.mybir as mybir
    import concourse.tile as tile
    from concourse import bacc

    F32 = mybir.dt.float32
    F32R = mybir.dt.float32r

    nc = bacc.Bacc("TRN2", target_bir_lowering=False, debug=False,
                   num_devices=N_CORES)
    x = nc.dram_tensor("x", [TOK, D], F32R, kind="ExternalInput")
    w1t = nc.dram_tensor("w1t", [128, ND, M], F32R, kind="ExternalInput")
    w3t = nc.dram_tensor("w3t", [128, 2, J], F32R, kind="ExternalInput")
    c2d = nc.dram_tensor("c2", [1, M], F32R, kind="ExternalInput")
    c3d = nc.dram_tensor("c3rep", [128, JT], F32, kind="ExternalInput")
    ident = nc.dram_tensor("ident", [128, 128], F32R, kind="ExternalInput")
    onesd = nc.dram_tensor("ones", [1, CHUNK], F32R, kind="ExternalInput")
    y = nc.dram_tensor("y", [TOK, J], F32, kind="ExternalOutput")

    with tile.TileContext(nc) as tc:
        with (
            tc.tile_pool(name="const", bufs=1) as const_pool,
            tc.tile_pool(name="xin", bufs=3) as xin_pool,
            tc.tile_pool(name="xt", bufs=1) as xt_pool,
            tc.tile_pool(name="gt", bufs=2) as gt_pool,
            tc.tile_pool(name="yout", bufs=2) as yout_pool,
            tc.tile_pool(name="tp_psum", bufs=3, space="PSUM") as tp_psum,
            tc.tile_pool(name="g_psum", bufs=2, space="PSUM") as g_psum,
            tc.tile_pool(name="y_psum", bufs=2, space="PSUM") as y_psum,
        ):
            w1_sb = const_pool.tile([128, ND, M], F32R)
            nc.sync.dma_start(w1_sb[:], w1t[:])
            w3_sb = const_pool.tile([128, 2, J], F32R)
            nc.sync.dma_start(w3_sb[:], w3t[:])
            c2_sb = const_pool.tile([1, M], F32R)
            nc.sync.dma_start(c2_sb[:], c2d[:])
            c3_sb = const_pool.tile([128, JT], F32)
            nc.sync.dma_start(c3_sb[:], c3d[:])
            id_sb = const_pool.tile([128, 128], F32R)
            nc.sync.dma_start(id_sb[:], ident[:])
            ones_sb = const_pool.tile([1, CHUNK], F32R)
            nc.sync.dma_start(ones_sb[:], onesd[:])

            for ch in range(NCH):
                # ---- load + transpose 512 tokens: xt[d_p, kt, t] ----
                xt = xt_pool.tile([128, ND, CHUNK], F32R)
                for tt in range(TPC):
                    row0 = (ch * TPC + tt) * 128
                    for h in range(2):  # halves of the 4096-wide row block
                        xin = xin_pool.tile([128, D // 2], F32R)
                        nc.sync.dma_start(
                            xin[:], x[row0:row0 + 128,
                                      h * (D // 2):(h + 1) * (D // 2)])
                        for q in range(4):  # 4 transposes per PSUM bank
                            pt = tp_psum.tile([128, 512], F32R)
                            for i in range(4):
                                k = q * 4 + i
                                nc.tensor.transpose(
                                    pt[:, i * 128:(i + 1) * 128],
                                    xin[:, k * 128:(k + 1) * 128], id_sb[:])
                            kt0 = h * (ND // 2) + q * 4
                            # strided copy into 4 kt rows of xt
                            dst = xt[:, kt0:kt0 + 4, tt * 128:(tt + 1) * 128]
                            if q % 2 == 0:
                                nc.vector.tensor_copy(dst, pt[:])
                            else:
                                nc.scalar.copy(dst, pt[:])

                # ---- MM1: G^T [m_p, t] (+ c2 outer-product bias) ----
                gt = gt_pool.tile([128, 2, CHUNK], F32R)
                for mt in range(2):
                    gp = g_psum.tile([128, CHUNK], F32)
                    for kt in range(ND):
                        nc.tensor.matmul(
                            gp[:],
                            w1_sb[:, kt, mt * 128:(mt + 1) * 128],
                            xt[:, kt, :],
                            start=(kt == 0), stop=False)
                    nc.tensor.matmul(
                        gp[:], c2_sb[:, mt * 128:(mt + 1) * 128], ones_sb[:],
                        start=False, stop=True)
                    nc.vector.tensor_copy(gt[:, mt, :], gp[:])

                # ---- MM2 + c3 bias + store ----
                for tt in range(TPC):
                    row0 = (ch * TPC + tt) * 128
                    yo = yout_pool.tile([128, J], F32)
                    for jt in range(NJ):
                        yp = y_psum.tile([128, JT], F32)
                        for mt in range(2):
                            nc.tensor.matmul(
                                yp[:],
                                gt[:, mt, tt * 128:(tt + 1) * 128],
                                w3_sb[:, mt, jt * JT:(jt + 1) * JT],
                                start=(mt == 0), stop=(mt == 1))
                        dst = yo[:, jt * JT:(jt + 1) * JT]
                        nc.vector.tensor_add(dst, yp[:], c3_sb[:])
                    nc.sync.dma_start(y[row0:row0 + 128, :], yo[:])
    nc.compile()
    return nc


def _fold_weights(w1, w2, w3, bias1, bias2, bias3):
    """Collapse the 3-stage Clos into W1f [4096,256], W3f [256,4096], c2, c3."""
    w1 = np.asarray(w1, np.float64)
    w2 = np.asarray(w2, np.float64)
    w3 = np.asarray(w3, np.float64)
    b1 = np.asarray(bias1, np.float64)
    b2 = np.asarray(bias2, np.float64)
    b3 = np.asarray(bias3, np.float64)

    w2s = w2.sum(axis=2)                                   # [64(r), 256(m)]
    W1f = (w1 * w2s[None, :, :]).reshape(D, M)             # [(n,r), m]
    c2 = b1 @ w2s + w2.shape[2] * b2                       # [256]
    W3f = np.transpose(w3, (0, 2, 1)).reshape(M, J)        # [m, (o,r)]
    c3 = np.tile(b3, JT // b3.shape[0])                    # [512], period 64
    return W1f, W3f, c2, c3


def _device_arrays(w1, w2, w3, bias1, bias2, bias3):
    W1f, W3f, c2, c3 = _fold_weights(w1, w2, w3, bias1, bias2, bias3)
    w1t = np.ascontiguousarray(
        W1f.reshape(ND, 128, M).transpose(1, 0, 2)).astype(np.float32)
    w3t = np.ascontiguousarray(
        W3f.reshape(2, 128, J).transpose(1, 0, 2)).astype(np.float32)
    c2a = c2.astype(np.float32).reshape(1, M)
    c3rep = np.ascontiguousarray(
        np.broadcast_to(c3.astype(np.float32), (128, JT)))
    ident = np.eye(128, dtype=np.float32)
    ones = np.ones((1, CHUNK), dtype=np.float32)
    return {"w1t": w1t, "w3t": w3t, "c2": c2a, "c3rep": c3rep, "ident": ident,
            "ones": ones}


def kernel(x, w1, w2, w3, bias1, bias2, bias3):
    from concourse.bass_utils import run_bass_kernel_spmd

    consts = _device_arrays(w1, w2, w3, bias1, bias2, bias3)
    x2d = np.ascontiguousarray(np.asarray(x, np.float32).reshape(TOK_TOTAL, D))

    if "nc" not in _CACHE:
        _CACHE["nc"] = _build_nc()
    nc = _CACHE["nc"]

    in_maps = [
        {"x": np.ascontiguousarray(x2d[i * TOK:(i + 1) * TOK]), **consts}
        for i in range(N_CORES)
    ]
    res = run_bass_kernel_spmd(nc, in_maps, core_ids=list(range(N_CORES)))
    y = np.concatenate([res.results[i]["y"] for i in range(N_CORES)], axis=0)
    return y.reshape(x.shape[0], x.shape[1], J)
